# revision 1
# baseline (speedup 1.0000x reference)
"""Trainium2 Bass kernel for sparse-graph GCN (nn_HGC_LRN_25237227832003).

Pipeline per the reference:
  phi = MLP(col indices)                      [S=4096, D=256]  (host, tiny)
  h0  = (mask @ phi) / cnt                    [N=20000, D]     (device matmul)
  3x: h = relu(segment_sum(h[row]*attr, col) @ W)              (device)

Sharding: rows (learners/destinations) split across 8 cores, 2500 each,
padded to 2560 = 20 blocks of 128. Edges are assigned to the core owning
their destination, sorted by destination. Aggregation per destination
block = dma_gather of source rows + one-hot selector matmuls accumulated
in PSUM. h is replicated each layer via AllGather (bf16).
"""
import sys
import os
import numpy as np

for _p in ("/opt/trn_rl_repo",):
    if _p not in sys.path:
        sys.path.append(_p)

import ml_dtypes

N, S, E, D = 20000, 4096, 640000, 256
NCORES = 8
RPC = 2500            # real rows per core
NBLK = 20             # dest blocks per core
RPC_PAD = NBLK * 128  # 2560
NPAD = RPC_PAD * NCORES
KCH = S // 128        # 32 stage-1 contraction chunks
NLAYERS = 3

_nc_cache = {}


def _build_nc(CH):
    import concourse.bass as bass
    import concourse.bacc as bacc
    import concourse.tile as tile
    import concourse.mybir as mybir

    bf16 = mybir.dt.bfloat16
    f32 = mybir.dt.float32
    i16 = mybir.dt.int16

    nc = bacc.Bacc("TRN2", target_bir_lowering=False, debug=False,
                   num_devices=NCORES)

    mask_d = nc.dram_tensor("mask_blk", [NBLK, 128, S], bf16, kind="ExternalInput")
    phi_d = nc.dram_tensor("phi_sb", [128, KCH * D], bf16, kind="ExternalInput")
    w_d = nc.dram_tensor("w_sb", [128, 2 * D], bf16, kind="ExternalInput")
    iota_d = nc.dram_tensor("iota_row", [128, 128], bf16, kind="ExternalInput")
    id_d = nc.dram_tensor("ident", [128, 128], bf16, kind="ExternalInput")
    eidx_d = nc.dram_tensor("eidx", [128, NBLK * CH * 8], i16, kind="ExternalInput")
    edloc_d = nc.dram_tensor("edloc", [128, NBLK * CH], f32, kind="ExternalInput")
    eattr_d = nc.dram_tensor("eattr", [128, NBLK * CH], f32, kind="ExternalInput")
    out_d = nc.dram_tensor("out_ext", [RPC_PAD, D], f32, kind="ExternalOutput")

    rg = [list(range(NCORES))]

    with tile.TileContext(nc) as tc:
        with tc.tile_pool(name="const", bufs=1) as cst, \
             tc.tile_pool(name="maskp", bufs=3) as maskp, \
             tc.tile_pool(name="gp", bufs=2) as gp, \
             tc.tile_pool(name="selp", bufs=6) as selp, \
             tc.tile_pool(name="hp", bufs=3) as hp, \
             tc.tile_pool(name="psum", bufs=2, space="PSUM") as ps, \
             tc.tile_pool(name="dram", bufs=1, space="DRAM") as dram:

            phi_t = cst.tile([128, KCH * D], bf16, tag="phi")
            nc.sync.dma_start(out=phi_t[:], in_=phi_d.ap())
            w_t = cst.tile([128, 2 * D], bf16, tag="w")
            nc.sync.dma_start(out=w_t[:], in_=w_d.ap())
            iota_t = cst.tile([128, 128], bf16, tag="iota")
            nc.sync.dma_start(out=iota_t[:], in_=iota_d.ap())
            id_t = cst.tile([128, 128], bf16, tag="ident")
            nc.sync.dma_start(out=id_t[:], in_=id_d.ap())
            eidx_t = cst.tile([128, NBLK * CH * 8], i16, tag="eidx")
            nc.sync.dma_start(out=eidx_t[:], in_=eidx_d.ap())
            edloc_t = cst.tile([128, NBLK * CH], f32, tag="edloc")
            nc.sync.dma_start(out=edloc_t[:], in_=edloc_d.ap())
            eattr_t = cst.tile([128, NBLK * CH], f32, tag="eattr")
            nc.sync.dma_start(out=eattr_t[:], in_=eattr_d.ap())

            ag_in = [dram.tile([RPC_PAD, D], bf16, tag=f"ag_in{l}",
                               name=f"ag_in{l}") for l in range(NLAYERS)]
            ag_out = [dram.tile([NPAD, D], bf16, tag=f"ag_out{l}",
                                name=f"ag_out{l}") for l in range(NLAYERS)]

            # ---- stage 1: h0 = scaled_mask.T @ phi ----
            for nb in range(NBLK):
                mt = maskp.tile([128, S], bf16, tag="mt")
                nc.sync.dma_start(out=mt[:], in_=mask_d.ap()[nb])
                acc = ps.tile([128, D], f32, space="PSUM", tag="acc")
                for sc in range(KCH):
                    nc.tensor.matmul(
                        out=acc[:],
                        lhsT=mt[:, sc * 128:(sc + 1) * 128],
                        rhs=phi_t[:, sc * D:(sc + 1) * D],
                        start=(sc == 0), stop=(sc == KCH - 1),
                    )
                h0 = hp.tile([128, D], bf16, tag="hn")
                nc.vector.tensor_copy(out=h0[:], in_=acc[:])
                nc.sync.dma_start(out=ag_in[0][nb * 128:(nb + 1) * 128, :], in_=h0[:])

            # ---- layers ----
            for l in range(NLAYERS):
                nc.gpsimd.collective_compute(
                    "AllGather", mybir.AluOpType.bypass, replica_groups=rg,
                    ins=[ag_in[l].opt()], outs=[ag_out[l].opt()],
                )
                for nb in range(NBLK):
                    g = gp.tile([128, CH, D], bf16, tag="g")
                    for j in range(CH // 4):
                        nc.gpsimd.dma_gather(
                            g[:, j * 4:(j + 1) * 4, :], ag_out[l][:, :],
                            eidx_t[:, nb * CH * 8 + j * 32:
                                   nb * CH * 8 + (j + 1) * 32],
                            512, 512, D,
                        )
                    acc = ps.tile([128, D], f32, space="PSUM", tag="acc")
                    for k in range(CH):
                        sel = selp.tile([128, 128], bf16, tag="sel")
                        nc.vector.tensor_scalar(
                            out=sel[:], in0=iota_t[:],
                            scalar1=edloc_t[:, nb * CH + k:nb * CH + k + 1],
                            scalar2=eattr_t[:, nb * CH + k:nb * CH + k + 1],
                            op0=mybir.AluOpType.is_equal,
                            op1=mybir.AluOpType.mult,
                        )
                        nc.tensor.matmul(
                            out=acc[:], lhsT=sel[:], rhs=g[:, k, :],
                            start=(k == 0), stop=(k == CH - 1),
                        )
                    abf = hp.tile([128, D], bf16, tag="abf")
                    nc.vector.tensor_copy(out=abf[:], in_=acc[:])
                    tp = ps.tile([128, D], bf16, space="PSUM", tag="tp")
                    for di in range(2):
                        nc.tensor.transpose(
                            out=tp[:, di * 128:(di + 1) * 128],
                            in_=abf[:, di * 128:(di + 1) * 128],
                            identity=id_t[:],
                        )
                    att = hp.tile([128, D], bf16, tag="att")
                    nc.vector.tensor_copy(out=att[:], in_=tp[:])
                    ops_t = ps.tile([128, D], f32, space="PSUM", tag="ops")
                    for di in range(2):
                        nc.tensor.matmul(
                            out=ops_t[:],
                            lhsT=att[:, di * 128:(di + 1) * 128],
                            rhs=w_t[:, di * D:(di + 1) * D],
                            start=(di == 0), stop=(di == 1),
                        )
                    if l < NLAYERS - 1:
                        hn = hp.tile([128, D], bf16, tag="hn")
                        nc.vector.tensor_scalar(
                            out=hn[:], in0=ops_t[:], scalar1=0.0, scalar2=None,
                            op0=mybir.AluOpType.max,
                        )
                        nc.sync.dma_start(
                            out=ag_in[l + 1][nb * 128:(nb + 1) * 128, :], in_=hn[:])
                    else:
                        ho = hp.tile([128, D], f32, tag="ho")
                        nc.vector.tensor_scalar(
                            out=ho[:], in0=ops_t[:], scalar1=0.0, scalar2=None,
                            op0=mybir.AluOpType.max,
                        )
                        nc.sync.dma_start(
                            out=out_d.ap()[nb * 128:(nb + 1) * 128, :], in_=ho[:])

    nc.compile()
    return nc


def _prep_inputs(init, edge_index, edge_attr, w1, b1, w2, b2, W):
    bf = ml_dtypes.bfloat16

    # phi = MLP(column indices), tiny — fp32 on host
    idx = np.arange(S, dtype=np.float32)[:, None]
    phi = np.maximum(idx @ np.asarray(w1, np.float32) + np.asarray(b1, np.float32),
                     0.0) @ np.asarray(w2, np.float32) + np.asarray(b2, np.float32)
    phi_sb = np.ascontiguousarray(
        phi.reshape(KCH, 128, D).transpose(1, 0, 2).reshape(128, KCH * D)
    ).astype(bf)

    Wf = np.asarray(W, np.float32)
    w_sb = np.ascontiguousarray(
        Wf.reshape(2, 128, D).transpose(1, 0, 2).reshape(128, 2 * D)).astype(bf)
    iota_row = np.tile(np.arange(128, dtype=np.float32), (128, 1)).astype(bf)
    ident = np.eye(128, dtype=np.float32).astype(bf)

    # ---- edges: assign to dest core, sort by (core, block, dloc) ----
    row = np.asarray(edge_index[0], np.int64)
    col = np.asarray(edge_index[1], np.int64)
    attr = np.asarray(edge_attr, np.float32)
    src_pad = (row // RPC) * RPC_PAD + (row % RPC)
    core = col // RPC
    dl_all = col % RPC
    blk = dl_all // 128
    dloc = dl_all % 128
    key = (core * NBLK + blk).astype(np.int64)
    order = np.lexsort((dloc, key))
    s_key = key[order]
    s_src = src_pad[order]
    s_dloc = dloc[order]
    s_attr = attr[order]
    counts = np.bincount(s_key, minlength=NCORES * NBLK)
    CH = int(np.ceil(counts.max() / 128))
    CH = ((CH + 3) // 4) * 4  # gathers run in 512-index groups
    EPB = CH * 128
    starts = np.zeros(NCORES * NBLK, np.int64)
    starts[1:] = np.cumsum(counts)[:-1]
    within = np.arange(E) - starts[s_key]
    flat = s_key * EPB + within

    srcp = np.zeros(NCORES * NBLK * EPB, np.int16)
    dlocp = np.zeros(NCORES * NBLK * EPB, np.float32)
    attrp = np.zeros(NCORES * NBLK * EPB, np.float32)
    srcp[flat] = s_src.astype(np.int16)
    dlocp[flat] = s_dloc
    attrp[flat] = s_attr
    srcp = srcp.reshape(NCORES, NBLK, EPB)
    dlocp = dlocp.reshape(NCORES, NBLK, EPB)
    attrp = attrp.reshape(NCORES, NBLK, EPB)

    # device layouts
    ii = np.arange(EPB)
    wrapped = np.zeros((NCORES, NBLK, 16, CH * 8), np.int16)
    wrapped[:, :, ii % 16, ii // 16] = srcp
    eidx = np.tile(wrapped, (1, 1, 8, 1))            # [8, 20, 128, CH*8]
    eidx = np.ascontiguousarray(
        eidx.transpose(0, 2, 1, 3).reshape(NCORES, 128, NBLK * CH * 8))
    # edge i=k*128+p -> [p, k]
    edloc = np.ascontiguousarray(
        dlocp.reshape(NCORES, NBLK, CH, 128).transpose(0, 3, 1, 2)
        .reshape(NCORES, 128, NBLK * CH))
    eattr = np.ascontiguousarray(
        attrp.reshape(NCORES, NBLK, CH, 128).transpose(0, 3, 1, 2)
        .reshape(NCORES, 128, NBLK * CH))

    # ---- scaled mask blocks per core ----
    init = np.asarray(init)
    in_maps = []
    for c in range(NCORES):
        rows_c = init[c * RPC:(c + 1) * RPC]
        m = (rows_c != 0).astype(np.float32)
        cnt = m.sum(axis=1)
        scal = m / np.maximum(cnt, 1.0)[:, None]
        sp = np.zeros((RPC_PAD, S), np.float32)
        sp[:RPC] = scal
        mask_blk = np.ascontiguousarray(
            sp.reshape(NBLK, 128, KCH, 128).transpose(0, 3, 2, 1)
            .reshape(NBLK, 128, S)).astype(bf)
        in_maps.append({
            "mask_blk": mask_blk,
            "phi_sb": phi_sb,
            "w_sb": w_sb,
            "iota_row": iota_row,
            "ident": ident,
            "eidx": eidx[c],
            "edloc": edloc[c],
            "eattr": eattr[c],
        })
    return in_maps, CH


def kernel(init, edge_index, edge_attr, w1, b1, w2, b2, W, _trace=False):
    from concourse.bass_utils import run_bass_kernel_spmd

    in_maps, CH = _prep_inputs(init, edge_index, edge_attr, w1, b1, w2, b2, W)
    if CH not in _nc_cache:
        _nc_cache[CH] = _build_nc(CH)
    nc = _nc_cache[CH]
    res = run_bass_kernel_spmd(nc, in_maps, core_ids=list(range(NCORES)),
                               trace=_trace)
    kernel.last_results = res
    full = np.empty((N, D), np.float32)
    for c in range(NCORES):
        full[c * RPC:(c + 1) * RPC] = res.results[c]["out_ext"][:RPC]
    return full



# revision 6
# speedup vs baseline: 14.7992x; 14.7992x over previous
"""Trainium2 Bass kernel for sparse-graph GCN (nn_HGC_LRN_25237227832003).

Pipeline per the reference:
  phi = MLP(col indices)                      [S=4096, D=256]  (host, tiny)
  h0  = (mask @ phi) / cnt                    [N=20000, D]     (device matmul)
  3x: h = relu(segment_sum(h[row]*attr, col) @ W)              (device)

Sharding: rows (learners/destinations) split across 8 cores, 2500 each,
padded to 2560 = 20 blocks of 128. Edges are assigned to the core owning
their destination, sorted by destination. Aggregation per destination
block = dma_gather of the block's source rows (1024-index calls — the
SWDGE descriptor ring holds ~1024 descriptors, so bigger calls hang) +
one-hot selector matmuls accumulated in PSUM. h is replicated each layer
via AllGather (bf16, Shared-scratchpad output = the HBM-HBM fast path).
"""
import sys
import os
import numpy as np

for _p in ("/opt/trn_rl_repo",):
    if _p not in sys.path:
        sys.path.append(_p)

import ml_dtypes

N, S, E, D = 20000, 4096, 640000, 256
NCORES = 8
RPC = 2500            # real rows per core
NBLK = 20             # dest blocks per core
RPC_PAD = NBLK * 128  # 2560
NPAD = RPC_PAD * NCORES
KCH = S // 128        # 32 stage-1 contraction chunks
NLAYERS = 3

_nc_cache = {}


def _build_nc(CH):
    import concourse.bass as bass
    import concourse.bacc as bacc
    import concourse.tile as tile
    import concourse.mybir as mybir

    bf16 = mybir.dt.bfloat16
    f32 = mybir.dt.float32
    i16 = mybir.dt.int16

    nc = bacc.Bacc("TRN2", target_bir_lowering=False, debug=False,
                   num_devices=NCORES)

    mask_d = nc.dram_tensor("mask_blk", [NBLK, 128, S], bf16, kind="ExternalInput")
    phi_d = nc.dram_tensor("phi_sb", [128, KCH * D], bf16, kind="ExternalInput")
    w_d = nc.dram_tensor("w_sb", [128, 2 * D], bf16, kind="ExternalInput")
    iota_d = nc.dram_tensor("iota_row", [128, 128], bf16, kind="ExternalInput")
    id_d = nc.dram_tensor("ident", [128, 128], bf16, kind="ExternalInput")
    eidx_d = nc.dram_tensor("eidx", [128, NBLK * CH * 8], i16, kind="ExternalInput")
    edloc_d = nc.dram_tensor("edloc", [128, NBLK * CH], f32, kind="ExternalInput")
    eattr_d = nc.dram_tensor("eattr", [128, NBLK * CH], f32, kind="ExternalInput")
    out_d = nc.dram_tensor("out_ext", [RPC_PAD, D], f32, kind="ExternalOutput")

    rg = [list(range(NCORES))]

    with tile.TileContext(nc) as tc:
        with tc.tile_pool(name="const", bufs=1) as cst, \
             tc.tile_pool(name="maskp", bufs=3) as maskp, \
             tc.tile_pool(name="gp", bufs=3) as gp, \
             tc.tile_pool(name="selp", bufs=6) as selp, \
             tc.tile_pool(name="hp", bufs=3) as hp, \
             tc.tile_pool(name="psum", bufs=2, space="PSUM") as ps, \
             tc.tile_pool(name="dram", bufs=1, space="DRAM") as dram:

            phi_t = cst.tile([128, KCH * D], bf16, tag="phi")
            nc.sync.dma_start(out=phi_t[:], in_=phi_d.ap())
            w_t = cst.tile([128, 2 * D], bf16, tag="w")
            nc.sync.dma_start(out=w_t[:], in_=w_d.ap())
            iota_t = cst.tile([128, 128], bf16, tag="iota")
            nc.sync.dma_start(out=iota_t[:], in_=iota_d.ap())
            id_t = cst.tile([128, 128], bf16, tag="ident")
            nc.sync.dma_start(out=id_t[:], in_=id_d.ap())
            eidx_t = cst.tile([128, NBLK * CH * 8], i16, tag="eidx")
            nc.sync.dma_start(out=eidx_t[:], in_=eidx_d.ap())
            edloc_t = cst.tile([128, NBLK * CH], f32, tag="edloc")
            nc.sync.dma_start(out=edloc_t[:], in_=edloc_d.ap())
            eattr_t = cst.tile([128, NBLK * CH], f32, tag="eattr")
            nc.sync.dma_start(out=eattr_t[:], in_=eattr_d.ap())

            ag_in = [dram.tile([RPC_PAD, D], bf16, tag=f"ag_in{l}",
                               name=f"ag_in{l}") for l in range(NLAYERS)]
            ag_out = [dram.tile([NPAD, D], bf16, tag=f"ag_out{l}",
                                name=f"ag_out{l}", addr_space="Shared")
                      for l in range(NLAYERS)]

            # ---- stage 1: h0 = scaled_mask.T @ phi ----
            for nb in range(NBLK):
                mt = maskp.tile([128, S], bf16, tag="mt")
                nc.sync.dma_start(out=mt[:], in_=mask_d.ap()[nb])
                acc = ps.tile([128, D], f32, space="PSUM", tag="acc")
                for sc in range(KCH):
                    nc.tensor.matmul(
                        out=acc[:],
                        lhsT=mt[:, sc * 128:(sc + 1) * 128],
                        rhs=phi_t[:, sc * D:(sc + 1) * D],
                        start=(sc == 0), stop=(sc == KCH - 1),
                    )
                h0 = hp.tile([128, D], bf16, tag="hn")
                nc.vector.tensor_copy(out=h0[:], in_=acc[:])
                nc.sync.dma_start(out=ag_in[0][nb * 128:(nb + 1) * 128, :], in_=h0[:])

            # ---- layers ----
            for l in range(NLAYERS):
                nc.gpsimd.collective_compute(
                    "AllGather", mybir.AluOpType.bypass, replica_groups=rg,
                    ins=[ag_in[l].opt()], outs=[ag_out[l].opt()],
                )
                for nb in range(NBLK):
                    # gather the block's source rows in 1024-index calls
                    # (SWDGE descriptor ring holds ~1024 descriptors)
                    g = gp.tile([128, CH, D], bf16, tag="g")
                    j = 0
                    while j < CH:
                        w = min(8, CH - j)
                        nc.gpsimd.dma_gather(
                            g[:, j:j + w, :], ag_out[l][:, :],
                            eidx_t[:, nb * CH * 8 + j * 8:
                                   nb * CH * 8 + (j + w) * 8],
                            w * 128, w * 128, D,
                        )
                        j += w
                    acc = ps.tile([128, D], f32, space="PSUM", tag="acc")
                    for k in range(CH):
                        sel = selp.tile([128, 128], bf16, tag="sel")
                        nc.vector.tensor_scalar(
                            out=sel[:], in0=iota_t[:],
                            scalar1=edloc_t[:, nb * CH + k:nb * CH + k + 1],
                            scalar2=eattr_t[:, nb * CH + k:nb * CH + k + 1],
                            op0=mybir.AluOpType.is_equal,
                            op1=mybir.AluOpType.mult,
                        )
                        nc.tensor.matmul(
                            out=acc[:], lhsT=sel[:], rhs=g[:, k, :],
                            start=(k == 0), stop=(k == CH - 1),
                        )
                    abf = hp.tile([128, D], bf16, tag="abf")
                    nc.vector.tensor_copy(out=abf[:], in_=acc[:])
                    tp = ps.tile([128, D], bf16, space="PSUM", tag="tp")
                    for di in range(2):
                        nc.tensor.transpose(
                            out=tp[:, di * 128:(di + 1) * 128],
                            in_=abf[:, di * 128:(di + 1) * 128],
                            identity=id_t[:],
                        )
                    att = hp.tile([128, D], bf16, tag="att")
                    nc.vector.tensor_copy(out=att[:], in_=tp[:])
                    ops_t = ps.tile([128, D], f32, space="PSUM", tag="ops")
                    for di in range(2):
                        nc.tensor.matmul(
                            out=ops_t[:],
                            lhsT=att[:, di * 128:(di + 1) * 128],
                            rhs=w_t[:, di * D:(di + 1) * D],
                            start=(di == 0), stop=(di == 1),
                        )
                    if l < NLAYERS - 1:
                        hn = hp.tile([128, D], bf16, tag="hn")
                        nc.vector.tensor_scalar(
                            out=hn[:], in0=ops_t[:], scalar1=0.0, scalar2=None,
                            op0=mybir.AluOpType.max,
                        )
                        nc.sync.dma_start(
                            out=ag_in[l + 1][nb * 128:(nb + 1) * 128, :], in_=hn[:])
                    else:
                        ho = hp.tile([128, D], f32, tag="ho")
                        nc.vector.tensor_scalar(
                            out=ho[:], in0=ops_t[:], scalar1=0.0, scalar2=None,
                            op0=mybir.AluOpType.max,
                        )
                        nc.sync.dma_start(
                            out=out_d.ap()[nb * 128:(nb + 1) * 128, :], in_=ho[:])

    nc.compile()
    return nc


def _prep_inputs(init, edge_index, edge_attr, w1, b1, w2, b2, W):
    bf = ml_dtypes.bfloat16

    # phi = MLP(column indices), tiny — fp32 on host
    idx = np.arange(S, dtype=np.float32)[:, None]
    phi = np.maximum(idx @ np.asarray(w1, np.float32) + np.asarray(b1, np.float32),
                     0.0) @ np.asarray(w2, np.float32) + np.asarray(b2, np.float32)
    phi_sb = np.ascontiguousarray(
        phi.reshape(KCH, 128, D).transpose(1, 0, 2).reshape(128, KCH * D)
    ).astype(bf)

    Wf = np.asarray(W, np.float32)
    w_sb = np.ascontiguousarray(
        Wf.reshape(2, 128, D).transpose(1, 0, 2).reshape(128, 2 * D)).astype(bf)
    iota_row = np.tile(np.arange(128, dtype=np.float32), (128, 1)).astype(bf)
    ident = np.eye(128, dtype=np.float32).astype(bf)

    # ---- edges: assign to dest core, sort by (core, block, dloc) ----
    row = np.asarray(edge_index[0], np.int64)
    col = np.asarray(edge_index[1], np.int64)
    attr = np.asarray(edge_attr, np.float32)
    src_pad = (row // RPC) * RPC_PAD + (row % RPC)
    core = col // RPC
    dl_all = col % RPC
    blk = dl_all // 128
    dloc = dl_all % 128
    key = (core * NBLK + blk).astype(np.int64)
    order = np.lexsort((dloc, key))
    s_key = key[order]
    s_src = src_pad[order]
    s_dloc = dloc[order]
    s_attr = attr[order]
    counts = np.bincount(s_key, minlength=NCORES * NBLK)
    CH = int(np.ceil(counts.max() / 128))
    CH = ((CH + 3) // 4) * 4
    EPB = CH * 128
    starts = np.zeros(NCORES * NBLK, np.int64)
    starts[1:] = np.cumsum(counts)[:-1]
    within = np.arange(E) - starts[s_key]
    flat = s_key * EPB + within

    srcp = np.zeros(NCORES * NBLK * EPB, np.int16)
    dlocp = np.zeros(NCORES * NBLK * EPB, np.float32)
    attrp = np.zeros(NCORES * NBLK * EPB, np.float32)
    srcp[flat] = s_src.astype(np.int16)
    dlocp[flat] = s_dloc
    attrp[flat] = s_attr
    srcp = srcp.reshape(NCORES, NBLK, EPB)
    dlocp = dlocp.reshape(NCORES, NBLK, EPB)
    attrp = attrp.reshape(NCORES, NBLK, EPB)

    # device layouts
    ii = np.arange(EPB)
    wrapped = np.zeros((NCORES, NBLK, 16, CH * 8), np.int16)
    wrapped[:, :, ii % 16, ii // 16] = srcp
    eidx = np.tile(wrapped, (1, 1, 8, 1))            # [8, 20, 128, CH*8]
    eidx = np.ascontiguousarray(
        eidx.transpose(0, 2, 1, 3).reshape(NCORES, 128, NBLK * CH * 8))
    # edge i=k*128+p -> [p, k]
    edloc = np.ascontiguousarray(
        dlocp.reshape(NCORES, NBLK, CH, 128).transpose(0, 3, 1, 2)
        .reshape(NCORES, 128, NBLK * CH))
    eattr = np.ascontiguousarray(
        attrp.reshape(NCORES, NBLK, CH, 128).transpose(0, 3, 1, 2)
        .reshape(NCORES, 128, NBLK * CH))

    # ---- scaled mask blocks per core ----
    init = np.asarray(init)
    in_maps = []
    for c in range(NCORES):
        rows_c = init[c * RPC:(c + 1) * RPC]
        m = (rows_c != 0).astype(np.float32)
        cnt = m.sum(axis=1)
        scal = m / np.maximum(cnt, 1.0)[:, None]
        sp = np.zeros((RPC_PAD, S), np.float32)
        sp[:RPC] = scal
        mask_blk = np.ascontiguousarray(
            sp.reshape(NBLK, 128, KCH, 128).transpose(0, 3, 2, 1)
            .reshape(NBLK, 128, S)).astype(bf)
        in_maps.append({
            "mask_blk": mask_blk,
            "phi_sb": phi_sb,
            "w_sb": w_sb,
            "iota_row": iota_row,
            "ident": ident,
            "eidx": eidx[c],
            "edloc": edloc[c],
            "eattr": eattr[c],
        })
    return in_maps, CH


def kernel(init, edge_index, edge_attr, w1, b1, w2, b2, W, _trace=False):
    from concourse.bass_utils import run_bass_kernel_spmd

    in_maps, CH = _prep_inputs(init, edge_index, edge_attr, w1, b1, w2, b2, W)
    if CH not in _nc_cache:
        _nc_cache[CH] = _build_nc(CH)
    nc = _nc_cache[CH]
    res = run_bass_kernel_spmd(nc, in_maps, core_ids=list(range(NCORES)),
                               trace=_trace)
    kernel.last_results = res
    full = np.empty((N, D), np.float32)
    for c in range(NCORES):
        full[c * RPC:(c + 1) * RPC] = res.results[c]["out_ext"][:RPC]
    return full
